# revision 45
# baseline (speedup 1.0000x reference)
"""Trainium2 Bass kernel for nn_Attention_72541997629647 (sparse varlen attention).

Computation (see problem reference):
  qkv = x @ w_qkv.T + b_qkv ; NeoX RoPE on q,k ; block-diagonal softmax
  attention from cu_seqlens segments ; out = (attn @ v) @ w_proj.T + b_proj

Sharding: tensor-parallel over heads. 16 heads / 8 cores = 2 heads per core.
Each core computes q/k/v for its 2 heads, runs block-diagonal attention, and
produces a partial projection output (full [DIM, S], transposed); the host
sums the 8 partials and adds b_proj, so the result is exact.

Device dataflow per core (all matmuls in float32r: full fp32 storage,
reduced-precision multiply at 4x the fp32 matmul rate):
  A) QKV: out_nat[s, 480] = xT-chunks.T @ w_chunks (+ bias via ones-row
     matmul); RoPE applied on the free dim (half-swap via negative-step AP,
     sign folded into the host-built sin table); q,k PE-transposed to
     [hd, S]; v kept natural with an appended ones column (denominator trick).
  B) per (head, segment, q-chunk): scoresT[k,q] = kT-block.T @ qT ; exp on
     ACT ; attn_extT[81, q] += v_ext.T @ exp accumulated over k-blocks; row 80
     is the softmax denominator. normalize = reciprocal + ones-matmul
     partition-broadcast + multiply.
  C) proj: outT[dim, s] += wpT-head.T @ attn_outT-head ; PSUM->SBUF copy on
     DVE; output written as fully-contiguous 1.25MB blocks (one dense
     descriptor chain per DMA, ~70us faster than 2KB-strided rows) and
     unscrambled on the host. b_proj is added host-side.
"""

import os
import sys

for _p in ("/opt/trn_rl_repo", "/root/.axon_site/_ro/trn_rl_repo"):
    if os.path.isdir(_p) and _p not in sys.path:
        sys.path.insert(0, _p)

import numpy as np

import concourse.bacc as bacc
import concourse.bass as bass
import concourse.mybir as mybir
import concourse.tile as tile
from concourse.bass_utils import run_bass_kernel_spmd
from contextlib import ExitStack

S = 3072
DIM = 1280
H = 16
HD = 80
NCORES = 8
HPC = H // NCORES          # heads per core = 2
QKDIM = 2 * HPC * HD       # 320 (q+k outdims per core)
ODIM = 3 * HPC * HD        # 480 (qkv outdims per core)
CDIM = HPC * HD            # 160 (attn channels per core)

F32 = mybir.dt.float32
F32R = mybir.dt.float32r
BF16 = mybir.dt.bfloat16
MM_DT = BF16               # matmul input dtype (1 cyc/row, halves DMA traffic;
                           # end-to-end max-rel error ~3e-3, budget is 2e-2)

_CACHE: dict = {}
B_MERGE = os.environ.get("B_MERGE", "0") == "1"


def _segments_from_cu(cu_seqlens: np.ndarray) -> tuple:
    """Contiguous runs of equal segment id, exactly as the reference's
    searchsorted-based mask defines them."""
    cu = np.asarray(cu_seqlens).astype(np.int64)
    seg = np.searchsorted(cu, np.arange(S), side="right") - 1
    change = np.nonzero(np.diff(seg))[0]
    starts = np.concatenate([[0], change + 1])
    ends = np.concatenate([change + 1, [S]])
    return tuple((int(a), int(b)) for a, b in zip(starts, ends))


def _build(segments, loop_n: int = 1) -> "bacc.Bacc":
    nc = bacc.Bacc("TRN2", target_bir_lowering=False, debug=False,
                   num_devices=NCORES)

    xblk_d = nc.dram_tensor("xblk", [S // 512, 5, 128, 2, 512], MM_DT,
                        kind="ExternalInput")
    wqkvT_d = nc.dram_tensor("wqkvT", [DIM, ODIM], MM_DT, kind="ExternalInput")
    bqkv_d = nc.dram_tensor("bqkv", [1, ODIM], MM_DT, kind="ExternalInput")
    cosb_d = nc.dram_tensor("cosb", [S // 512, 128, 4, HD], F32,
                        kind="ExternalInput")
    sinb_d = nc.dram_tensor("sinb", [S // 512, 128, 4, HD], F32,
                        kind="ExternalInput")
    wpT_d = nc.dram_tensor("wpT", [CDIM, DIM], MM_DT, kind="ExternalInput")
    ident_d = nc.dram_tensor("ident", [128, 128], MM_DT, kind="ExternalInput")
    ones_d = nc.dram_tensor("onesrow", [1, 128], MM_DT, kind="ExternalInput")
    vpad_d = nc.dram_tensor("vpad", [17], MM_DT, kind="ExternalInput")
    # boundary-block 0/1 masks (segments not aligned to the 128 grid);
    # order must match the (head-agnostic) traversal below.
    bpairs = []
    for (s0, s1) in segments:
        for j in range(s0 // 128, -(-s1 // 128)):
            r0, r1 = max(0, s0 - 128 * j), min(128, s1 - 128 * j)
            if r0 > 0 or r1 < 128:
                bpairs.append((j, r0, r1))
    nbm = len(bpairs)
    bmask_d = (nc.dram_tensor("bmask", [nbm, 128], MM_DT, kind="ExternalInput")
               if nbm else None)
    outb_d = nc.dram_tensor("outb", [S // 512, 2, 128, 5, 512], MM_DT,
                        kind="ExternalOutput")

    NT = S // 128   # 24 s-tiles
    NSS = S // 512  # 6 s-superchunks

    with tile.TileContext(nc) as tc, ExitStack() as ctx:
        if loop_n > 1:  # benchmarking only: repeat the whole body on-device
            ctx.enter_context(tc.For_i(0, loop_n, 1))
        per = ctx.enter_context(tc.tile_pool(name="persist", bufs=1))

        # small constants first so nothing cheap blocks the pipeline
        bqkv_sb = per.tile([1, ODIM], MM_DT, tag="bqkv")
        nc.sync.dma_start(out=bqkv_sb, in_=bqkv_d[:, :])
        ident_sb = per.tile([128, 128], MM_DT, tag="ident")
        nc.sync.dma_start(out=ident_sb, in_=ident_d[:, :])
        ones_sb = per.tile([1, 128], MM_DT, tag="ones")
        nc.sync.dma_start(out=ones_sb, in_=ones_d[:, :])
        # per-d-chunk qkv weights and per-superchunk rope tables: split so the
        # first matmul/rope can start after a fraction of the weight traffic
        wqkv_sb = [per.tile([128, ODIM], MM_DT, tag=f"wqkv{d}", name=f"wqkv{d}")
                   for d in range(10)]
        cos_sb = [per.tile([128, 4, HD], F32, tag=f"cos{ss}", name=f"cos{ss}")
                  for ss in range(NSS)]
        sin_sb = [per.tile([128, 4, HD], F32, tag=f"sin{ss}", name=f"sin{ss}")
                  for ss in range(NSS)]
        wp_sb = [per.tile([HD, DIM], MM_DT, tag=f"wp{h}", name=f"wp{h}") for h in range(HPC)]
        for h in range(HPC):
            nc.sync.dma_start(out=wp_sb[h], in_=wpT_d[h * HD:(h + 1) * HD, :])

        # v extended to 97 cols: 80 v-dims, 16 zero pad, ones col at 96 so the
        # denominator lands on a 32-aligned PSUM partition. Split per 512-s
        # superchunk so attention can start before all of phase A finishes.
        VEXT = 97
        v_sb = [[per.tile([128, 4, VEXT], MM_DT, tag=f"v{h}_{ss}",
                          name=f"v{h}_{ss}") for ss in range(NSS)]
                for h in range(HPC)]
        qkT = [[per.tile([HD, 512], MM_DT, tag=f"qkT{j}_{ss}",
                         name=f"qkT{j}_{ss}") for ss in range(NSS)]
               for j in range(2 * HPC)]
        att_o = [[per.tile([HD, 512], MM_DT, tag=f"atto{h}_{ss}",
                           name=f"atto{h}_{ss}") for ss in range(NSS)]
                 for h in range(HPC)]

        # one shared PSUM pool (8 bank-sized slots shared by every phase so
        # the scheduler can overlap A/B/C), plus top-level SBUF pools
        psp = ctx.enter_context(tc.tile_pool(name="ps", bufs=8, space="PSUM"))
        xtp = ctx.enter_context(tc.tile_pool(name="xt", bufs=10))
        stgp = ctx.enter_context(tc.tile_pool(name="stg", bufs=6))
        ropep = ctx.enter_context(tc.tile_pool(name="ropet", bufs=4))
        qkrop = ctx.enter_context(tc.tile_pool(name="qkro", bufs=4))
        expp = ctx.enter_context(tc.tile_pool(name="expp", bufs=14))
        smp = ctx.enter_context(tc.tile_pool(name="smalls", bufs=4))
        outp = ctx.enter_context(tc.tile_pool(name="outp", bufs=2))

        if nbm:
            bmask_sb = per.tile([128, nbm], MM_DT, tag="bmask")
            nc.sync.dma_start(out=bmask_sb,
                              in_=bmask_d.ap().rearrange("n p -> p n"))
            bidx = {(j, r0, r1): i for i, (j, r0, r1) in enumerate(bpairs)}

        # ---------------- phase bodies (emitted interleaved below) --------
        nh = 2 * HPC  # 4 roped qk tensor-heads

        def a_quanta(ss):
            """QKV + RoPE + transposes for s-superchunk ss, as a list of
            small emission quanta so B/C work can interleave between them."""
            st = {}

            def header():
                xts = []
                for dp in range(5):
                    if ss == 0:
                        for d in (2 * dp, 2 * dp + 1):
                            nc.sync.dma_start(
                                out=wqkv_sb[d],
                                in_=wqkvT_d[128 * d:128 * (d + 1), :])
                    xt = xtp.tile([128, 2, 512], MM_DT, tag="xt", name="xt")
                    nc.sync.dma_start(out=xt, in_=xblk_d[ss, dp])
                    xts.append(xt)
                nc.sync.dma_start(out=cos_sb[ss], in_=cosb_d[ss])
                nc.sync.dma_start(out=sin_sb[ss], in_=sinb_d[ss])
                for h in range(HPC):
                    nc.sync.dma_start(
                        out=v_sb[h][ss][:, :, HD:VEXT],
                        in_=bass.AP(tensor=vpad_d, offset=0,
                                    ap=[[0, 128], [0, 4], [1, VEXT - HD]]))
                st["xts"] = xts
                # all 16 sub-transposes packed into 2 PSUM banks (not 4) so
                # phase B keeps enough free banks to run ahead
                st["tp"] = [psp.tile([HD, 4, 2, 128], MM_DT, tag="ps",
                                     name="tpps") for _ in range(HPC)]

            def half_qkT_copy(half):
                # copy transposed qk halves out as soon as both their subs
                # are emitted: B quanta touching only this half can then
                # start without waiting for the rest of the superchunk
                for j in range(2 * HPC):
                    nc.vector.tensor_copy(
                        qkT[j][ss][:, 256 * half:256 * (half + 1)]
                        .rearrange("p (s c) -> p s c", s=2),
                        st["tp"][j // 2][:, 2 * half:2 * half + 2, j % 2, :])

            def transposes(sub):
                # transpose roped q,k of `sub` to [hd, s]; emitted one sub
                # late so the rope chain hides behind the next sub's matmuls
                ro = st.pop(f"ro{sub}")
                for j in range(2 * HPC):
                    nc.tensor.transpose(
                        st["tp"][j // 2][:, sub, j % 2, :],
                        ro[:, HD * j:HD * (j + 1)], ident_sb)
                if sub == 1:
                    half_qkT_copy(0)
                elif sub == 3:
                    half_qkT_copy(1)

            def sub_q(sub):
                def f():
                    xts = st["xts"]
                    qp = psp.tile([128, ODIM], F32, tag="ps", name="qkvps")
                    for d in range(10):
                        nc.tensor.matmul(
                            qp[:, :],
                            lhsT=xts[d // 2][:, d % 2,
                                             128 * sub:128 * (sub + 1)],
                            rhs=wqkv_sb[d], start=(d == 0), stop=False)
                    nc.tensor.matmul(qp[:, :], lhsT=ones_sb[:, :],
                                     rhs=bqkv_sb[:, :], start=False, stop=True)
                    if sub > 0:
                        transposes(sub - 1)

                    # single PSUM->SBUF staging copy (DVE); rope + v copies
                    # then run on the otherwise-idle Pool engine (Pool cannot
                    # touch PSUM, so the stage hop is what unlocks it)
                    stg = stgp.tile([128, ODIM], F32, tag="stg", name="stg")
                    nc.vector.tensor_copy(stg, qp)

                    # RoPE over q,k: out = t*cos + halfswap(t)*sinsgn
                    m1 = ropep.tile([128, QKDIM], F32, tag="m1")
                    m2 = ropep.tile([128, QKDIM], F32, tag="m2")
                    qk_h = stg[:, 0:QKDIM].rearrange("p (h d) -> p h d", h=nh)
                    cos_b = cos_sb[ss][:, sub:sub + 1, :].to_broadcast(
                        [128, nh, HD])
                    nc.gpsimd.tensor_mul(
                        m1.rearrange("p (h d) -> p h d", h=nh), qk_h, cos_b)
                    swap = stg[:, 0:QKDIM].rearrange(
                        "p (h x d) -> p h x d", h=nh, x=2)[:, :, ::-1, :]
                    sin_b = sin_sb[ss][:, sub:sub + 1, :].rearrange(
                        "p t (x d) -> p (t x) d", x=2)[:, None, :, :] \
                        .to_broadcast([128, nh, 2, HD // 2])
                    nc.gpsimd.tensor_mul(
                        m2.rearrange("p (h x d) -> p h x d", h=nh, x=2),
                        swap, sin_b)
                    ro = qkrop.tile([128, QKDIM], MM_DT, tag="qkro")
                    with nc.allow_low_precision("bf16 matmul inputs"):
                        nc.gpsimd.tensor_add(ro, m1, m2)
                    st[f"ro{sub}"] = ro

                    # v natural copy (its bias already in psum)
                    for h in range(HPC):
                        with nc.allow_low_precision("bf16 matmul inputs"):
                            nc.gpsimd.tensor_copy(
                                v_sb[h][ss][:, sub, 0:HD],
                                stg[:, QKDIM + HD * h:QKDIM + HD * (h + 1)])
                return f

            def last_tp():
                transposes(3)

            return [header] + [sub_q(s) for s in range(4)] + [last_tp]

        # C-superchunk readiness bookkeeping (emission-order only; runtime
        # ordering is enforced by tile dependencies)
        from collections import deque
        ready: deque = deque()
        need = {sc: 0 for sc in range(NSS)}
        got = {sc: 0 for sc in range(NSS)}

        def seg_qchunks(seg):
            s0, s1 = seg
            g = (s0 // 512) * 512
            out = []
            while g < s1:
                q0, q1 = max(s0, g), min(s1, g + 512)
                if q1 > q0:
                    out.append((q0, q1))
                g += 512
            return out

        def b_quantum(seg, h, chunks):
            """attention for one (segment, head) over `chunks` q-chunks:
            chunks processed together share each k/v stationary block (one
            LdWeights per block); av of block j-1 is emitted between scores
            and av of block j so exp latency hides behind PE work."""
            s0, s1 = seg
            jb0, jb1 = s0 // 128, -(-s1 // 128)
            qcs = []
            for (q0, q1) in chunks:
                qn = q1 - q0
                ss_q, c0 = q0 // 512, q0 % 512
                # matmul moving dim kept 4-aligned: widen the compute window
                # to 4-aligned columns (scratch cols unread)
                qa0 = q0 - (q0 % 4)
                qa1 = min(512 * (ss_q + 1), q1 + ((-q1) % 4))
                qcs.append((q0, q1, qn, ss_q, c0,
                            qa1 - qa0, q0 - qa0, qa0 % 512))
            st = {"ap": [], "den": []}

            def body():
                blocks = list(range(jb0, jb1))
                for _ in qcs:
                    st["ap"].append(
                        psp.tile([VEXT, 512], F32, tag="ps", name="attps"))
                # scores for every (block, chunk) pair first — chunks of one
                # block share the kT LdWeights — then one contiguous av
                # accumulation group per chunk (PSUM accumulation groups must
                # not interleave with each other on the in-order PE).
                exs = {}
                for j in blocks:
                    kTt = qkT[HPC + h][j // 4]
                    for ci, (q0, q1, qn, ss_q, c0, qna, off, ca0) \
                            in enumerate(qcs):
                        sc = psp.tile([128, 512], F32, tag="ps",
                                      name="scps")
                        nc.tensor.matmul(
                            sc[:, :qna],
                            lhsT=kTt[:, 128 * (j % 4):128 * (j % 4 + 1)],
                            rhs=qkT[h][ss_q][:, ca0:ca0 + qna],
                            start=True, stop=True)
                        ex = expp.tile([128, 512], MM_DT, tag="expp")
                        with nc.allow_low_precision("bf16 matmul inputs"):
                            nc.scalar.activation(
                                ex[:, :qna], sc[:, :qna],
                                mybir.ActivationFunctionType.Exp)
                        r0, r1 = max(0, s0 - 128 * j), min(128, s1 - 128 * j)
                        if r0 > 0 or r1 < 128:
                            # zero out-of-segment rows of this block
                            mi = bidx[(j, r0, r1)]
                            with nc.allow_low_precision("f32r inputs"):
                                nc.vector.tensor_mul(
                                    ex[:, :qna], ex[:, :qna],
                                    bmask_sb[:, mi:mi + 1]
                                    .to_broadcast([128, qna]))
                        exs[(j, ci)] = ex
                for ci, (q0, q1, qn, ss_q, c0, qna, off, ca0) \
                        in enumerate(qcs):
                    for j in blocks:
                        nc.tensor.matmul(
                            st["ap"][ci][:, :qna],
                            lhsT=v_sb[h][j // 4][:, j % 4, :],
                            rhs=exs[(j, ci)][:, :qna],
                            start=(j == blocks[0]),
                            stop=(j == blocks[-1]))
                # the denominator rows leave PSUM right away; the rest of
                # the normalize tail is emitted later (after other PE work)
                for ci, (q0, q1, qn, ss_q, c0, qna, off, ca0) in \
                        enumerate(qcs):
                    den = smp.tile([1, 512], MM_DT, tag="den", name="den")
                    with nc.allow_low_precision("bf16 matmul inputs"):
                        nc.vector.tensor_copy(den[:, :qna],
                                              st["ap"][ci][96:97, :qna])
                    st["den"].append(den)

            def tail():
                recs = []
                for ci, (q0, q1, qn, ss_q, c0, qna, off, ca0) in \
                        enumerate(qcs):
                    bc = psp.tile([HD, 512], F32, tag="ps", name="bcps")
                    nc.tensor.matmul(bc[:, :qna], lhsT=ones_sb[:, 0:HD],
                                     rhs=st["den"][ci][:, :qna],
                                     start=True, stop=True)
                    rec = smp.tile([HD, 512], F32, tag="rec", name="rec")
                    nc.vector.reciprocal(rec[:, :qna], bc[:, :qna])
                    recs.append(rec)
                for ci, (q0, q1, qn, ss_q, c0, qna, off, ca0) in \
                        enumerate(qcs):
                    with nc.allow_low_precision("f32r matmul inputs"):
                        nc.vector.tensor_mul(
                            att_o[h][ss_q][:, c0:c0 + qn],
                            st["ap"][ci][0:HD, off:off + qn],
                            recs[ci][:, off:off + qn])
                    got[ss_q] += 1
                    if got[ss_q] == need[ss_q]:
                        ready.extend(c_quanta(ss_q))
            return body, tail

        def c_quanta(sc_):
            """projection for output s-superchunk sc_, one quantum per
            128-dim half. b_proj is added host-side after the partial sum."""
            def mk(mh):
                def f():
                    ob = outp.tile([128, 5, 512], MM_DT, tag="outp")
                    for mm_ in range(5):
                        m = 5 * mh + mm_
                        pp = psp.tile([128, 512], F32, tag="ps", name="prps")
                        for h in range(HPC):
                            nc.tensor.matmul(
                                pp[:, :],
                                lhsT=wp_sb[h][:, 128 * m:128 * (m + 1)],
                                rhs=att_o[h][sc_],
                                start=(h == 0), stop=(h == HPC - 1))
                        with nc.allow_low_precision("bf16 output partials"):
                            if mm_ % 2 == 0:
                                nc.scalar.copy(ob[:, mm_, :], pp)
                            else:
                                nc.vector.tensor_copy(ob[:, mm_, :], pp)
                    nc.sync.dma_start(out=outb_d[sc_, mh], in_=ob)
                return f
            return [mk(0), mk(1)]

        # ---- interleaved driver: A emitted in sub-sized quanta; pending
        # ---- B/C quanta drain between them so PE never sits on one phase.
        # ---- B normalize tails are deferred one quantum (the next body's
        # ---- matmuls hide the den-copy latency on the in-order PE stream).
        for seg in segments:
            for (q0, q1) in seg_qchunks(seg):
                need[q0 // 512] += HPC
        pend = [None]

        def make_b(seg, h, chunks):
            body, tail = b_quantum(seg, h, chunks)

            def run():
                body()
                if pend[0] is not None:
                    pend[0]()
                pend[0] = tail
            return run

        segs_left = sorted(segments, key=lambda s: s[1])
        for ss in range(NSS):
            for q in a_quanta(ss):
                q()
                for _ in range(2 if len(ready) > 3 else 1):
                    if ready:
                        ready.popleft()()
            done_to = 512 * (ss + 1)
            while segs_left and segs_left[0][1] <= done_to:
                seg = segs_left.pop(0)
                for h in range(HPC):
                    if B_MERGE:
                        ready.append(make_b(seg, h, seg_qchunks(seg)))
                    else:
                        for qc in seg_qchunks(seg):
                            ready.append(make_b(seg, h, [qc]))
        assert not segs_left
        while ready or pend[0] is not None:
            if ready:
                ready.popleft()()
            else:
                t, pend[0] = pend[0], None
                t()

    nc.compile()
    return nc


def _prep_inputs(x, cu_seqlens, rotary_pos_emb, w_qkv, b_qkv, w_proj, b_proj):
    """Host-side shard prep. Returns per-core input dicts."""
    import ml_dtypes
    mm_np = ml_dtypes.bfloat16
    scale = np.float32(1.0 / np.sqrt(np.float32(HD)))
    xT = np.ascontiguousarray(np.asarray(x, np.float32).T)
    w_qkv = np.asarray(w_qkv, np.float32)
    b_qkv = np.asarray(b_qkv, np.float32)
    w_proj = np.asarray(w_proj, np.float32)
    b_proj = np.asarray(b_proj, np.float32)
    rot = np.asarray(rotary_pos_emb, np.float32)

    cosw = np.concatenate([np.cos(rot), np.cos(rot)], axis=1).astype(np.float32)
    sinw = np.concatenate([-np.sin(rot), np.sin(rot)], axis=1).astype(np.float32)
    # blocked layouts so every device DMA reads one dense contiguous region:
    # xblk[ss,dp,p,c,n] = xT[256dp+128c+p, 512ss+n]; cosb[ss,p,t,d] likewise
    xblk = np.ascontiguousarray(
        xT.reshape(5, 2, 128, 6, 512).transpose(3, 0, 2, 1, 4)).astype(mm_np)
    cosb = np.ascontiguousarray(
        cosw.reshape(6, 4, 128, HD).transpose(0, 2, 1, 3))
    sinb = np.ascontiguousarray(
        sinw.reshape(6, 4, 128, HD).transpose(0, 2, 1, 3))
    ident = np.eye(128, dtype=mm_np)
    onesrow = np.ones((1, 128), dtype=mm_np)
    vpad = np.zeros(17, dtype=mm_np)
    vpad[16] = 1.0
    segments = _segments_from_cu(cu_seqlens)
    bmask_rows = []
    for (s0, s1) in segments:
        for j in range(s0 // 128, -(-s1 // 128)):
            r0, r1 = max(0, s0 - 128 * j), min(128, s1 - 128 * j)
            if r0 > 0 or r1 < 128:
                row = np.zeros(128, dtype=mm_np)
                row[r0:r1] = 1.0
                bmask_rows.append(row)
    bmask = np.stack(bmask_rows) if bmask_rows else None

    in_maps = []
    for c in range(NCORES):
        heads = [HPC * c + i for i in range(HPC)]
        idx = []
        for base in (0, DIM, 2 * DIM):           # q, k, v row blocks
            for h in heads:
                idx.extend(range(base + h * HD, base + (h + 1) * HD))
        w_c = w_qkv[idx, :].copy()
        b_c = b_qkv[idx].copy()
        w_c[:QKDIM // 2] *= scale                # scale q by 1/sqrt(HD)
        b_c[:QKDIM // 2] *= scale
        cdims = []
        for h in heads:
            cdims.extend(range(h * HD, (h + 1) * HD))
        wpT = np.ascontiguousarray(w_proj[:, cdims].T).astype(mm_np)  # [CDIM, DIM]
        in_maps.append({
            "xblk": xblk,
            "wqkvT": np.ascontiguousarray(w_c.T).astype(mm_np),
            "bqkv": np.ascontiguousarray(b_c[None, :]).astype(mm_np),
            "cosb": cosb,
            "sinb": sinb,
            "wpT": wpT,
            "ident": ident,
            "onesrow": onesrow,
            "vpad": vpad,
        })
        if bmask is not None:
            in_maps[-1]["bmask"] = bmask
    return in_maps


def run(inputs: dict, trace: bool = False):
    """Build (cached), run on 8 cores, return (out [S, DIM] fp32, results)."""
    segments = _segments_from_cu(inputs["cu_seqlens"])
    key = (segments, str(MM_DT))
    if key not in _CACHE:
        _CACHE[key] = _build(segments)
    nc = _CACHE[key]
    in_maps = _prep_inputs(
        inputs["x"], inputs["cu_seqlens"], inputs["rotary_pos_emb"],
        inputs["w_qkv"], inputs["b_qkv"], inputs["w_proj"], inputs["b_proj"])
    res = run_bass_kernel_spmd(nc, in_maps, core_ids=list(range(NCORES)),
                               trace=trace)
    acc = np.zeros((DIM, S), np.float64)
    for r in res.results:
        # blocked [sc, mh, p, c, n] -> [dim = 640*mh+128*c+p, s = 512*sc+n]
        acc += r["outb"].astype(np.float32) \
            .transpose(1, 3, 2, 0, 4).reshape(DIM, S)
    acc += np.asarray(inputs["b_proj"], np.float64)[:, None]
    out = np.ascontiguousarray(acc.T.astype(np.float32))
    return out, res


def kernel(**inputs) -> np.ndarray:
    out, _ = run(inputs, trace=False)
    return out



# revision 46
# speedup vs baseline: 1.5064x; 1.5064x over previous
"""Trainium2 Bass kernel for nn_Attention_72541997629647 (sparse varlen attention).

Computation (see problem reference):
  qkv = x @ w_qkv.T + b_qkv ; NeoX RoPE on q,k ; block-diagonal softmax
  attention from cu_seqlens segments ; out = (attn @ v) @ w_proj.T + b_proj

Sharding: tensor-parallel over heads. 16 heads / 8 cores = 2 heads per core.
Each core computes q/k/v for its 2 heads, runs block-diagonal attention, and
produces a partial projection output (full [DIM, S], transposed); the host
sums the 8 partials and adds b_proj, so the result is exact.

Device dataflow per core (all matmuls in float32r: full fp32 storage,
reduced-precision multiply at 4x the fp32 matmul rate):
  A) QKV: out_nat[s, 480] = xT-chunks.T @ w_chunks (+ bias via ones-row
     matmul); RoPE applied on the free dim (half-swap via negative-step AP,
     sign folded into the host-built sin table); q,k PE-transposed to
     [hd, S]; v kept natural with an appended ones column (denominator trick).
  B) per (head, segment, q-chunk): scoresT[k,q] = kT-block.T @ qT ; exp on
     ACT ; attn_extT[81, q] += v_ext.T @ exp accumulated over k-blocks; row 80
     is the softmax denominator. normalize = reciprocal + ones-matmul
     partition-broadcast + multiply.
  C) proj: outT[dim, s] += wpT-head.T @ attn_outT-head ; PSUM->SBUF copy on
     DVE; output written as fully-contiguous 1.25MB blocks (one dense
     descriptor chain per DMA, ~70us faster than 2KB-strided rows) and
     unscrambled on the host. b_proj is added host-side.
"""

import os
import sys

for _p in ("/opt/trn_rl_repo", "/root/.axon_site/_ro/trn_rl_repo"):
    if os.path.isdir(_p) and _p not in sys.path:
        sys.path.insert(0, _p)

import numpy as np

import concourse.bacc as bacc
import concourse.bass as bass
import concourse.mybir as mybir
import concourse.tile as tile
from concourse.bass_utils import run_bass_kernel_spmd
from contextlib import ExitStack

S = 3072
DIM = 1280
H = 16
HD = 80
NCORES = 8
HPC = H // NCORES          # heads per core = 2
QKDIM = 2 * HPC * HD       # 320 (q+k outdims per core)
ODIM = 3 * HPC * HD        # 480 (qkv outdims per core)
CDIM = HPC * HD            # 160 (attn channels per core)

F32 = mybir.dt.float32
F32R = mybir.dt.float32r
BF16 = mybir.dt.bfloat16
MM_DT = BF16               # matmul input dtype (1 cyc/row, halves DMA traffic;
                           # end-to-end max-rel error ~3e-3, budget is 2e-2)

_CACHE: dict = {}
B_MERGE = os.environ.get("B_MERGE", "0") == "1"


def _segments_from_cu(cu_seqlens: np.ndarray) -> tuple:
    """Contiguous runs of equal segment id, exactly as the reference's
    searchsorted-based mask defines them."""
    cu = np.asarray(cu_seqlens).astype(np.int64)
    seg = np.searchsorted(cu, np.arange(S), side="right") - 1
    change = np.nonzero(np.diff(seg))[0]
    starts = np.concatenate([[0], change + 1])
    ends = np.concatenate([change + 1, [S]])
    return tuple((int(a), int(b)) for a, b in zip(starts, ends))


def _build(segments, loop_n: int = 1) -> "bacc.Bacc":
    nc = bacc.Bacc("TRN2", target_bir_lowering=False, debug=False,
                   num_devices=NCORES)

    xblk_d = nc.dram_tensor("xblk", [S // 512, 5, 128, 2, 512], MM_DT,
                        kind="ExternalInput")
    wqkvT_d = nc.dram_tensor("wqkvT", [DIM, ODIM], MM_DT, kind="ExternalInput")
    bqkv_d = nc.dram_tensor("bqkv", [1, ODIM], MM_DT, kind="ExternalInput")
    cosb_d = nc.dram_tensor("cosb", [S // 512, 128, 4, HD], F32,
                        kind="ExternalInput")
    sinb_d = nc.dram_tensor("sinb", [S // 512, 128, 4, HD], F32,
                        kind="ExternalInput")
    wpT_d = nc.dram_tensor("wpT", [CDIM, DIM], MM_DT, kind="ExternalInput")
    ident_d = nc.dram_tensor("ident", [128, 128], MM_DT, kind="ExternalInput")
    ones_d = nc.dram_tensor("onesrow", [1, 128], MM_DT, kind="ExternalInput")
    vpad_d = nc.dram_tensor("vpad", [17], MM_DT, kind="ExternalInput")
    # boundary-block 0/1 masks (segments not aligned to the 128 grid);
    # order must match the (head-agnostic) traversal below.
    bpairs = []
    for (s0, s1) in segments:
        for j in range(s0 // 128, -(-s1 // 128)):
            r0, r1 = max(0, s0 - 128 * j), min(128, s1 - 128 * j)
            if r0 > 0 or r1 < 128:
                bpairs.append((j, r0, r1))
    nbm = len(bpairs)
    bmask_d = (nc.dram_tensor("bmask", [nbm, 128], MM_DT, kind="ExternalInput")
               if nbm else None)
    outb_d = nc.dram_tensor("outb", [S // 512, 2, 128, 5, 512], MM_DT,
                        kind="ExternalOutput")

    NT = S // 128   # 24 s-tiles
    NSS = S // 512  # 6 s-superchunks

    with tile.TileContext(nc) as tc, ExitStack() as ctx:
        if loop_n > 1:  # benchmarking only: repeat the whole body on-device
            ctx.enter_context(tc.For_i(0, loop_n, 1))
        per = ctx.enter_context(tc.tile_pool(name="persist", bufs=1))

        # small constants first so nothing cheap blocks the pipeline
        bqkv_sb = per.tile([1, ODIM], MM_DT, tag="bqkv")
        nc.sync.dma_start(out=bqkv_sb, in_=bqkv_d[:, :])
        ident_sb = per.tile([128, 128], MM_DT, tag="ident")
        nc.sync.dma_start(out=ident_sb, in_=ident_d[:, :])
        ones_sb = per.tile([1, 128], MM_DT, tag="ones")
        nc.sync.dma_start(out=ones_sb, in_=ones_d[:, :])
        # per-d-chunk qkv weights and per-superchunk rope tables: split so the
        # first matmul/rope can start after a fraction of the weight traffic
        wqkv_sb = [per.tile([128, ODIM], MM_DT, tag=f"wqkv{d}", name=f"wqkv{d}")
                   for d in range(10)]
        cos_sb = [per.tile([128, 4, HD], F32, tag=f"cos{ss}", name=f"cos{ss}")
                  for ss in range(NSS)]
        sin_sb = [per.tile([128, 4, HD], F32, tag=f"sin{ss}", name=f"sin{ss}")
                  for ss in range(NSS)]
        wp_sb = [per.tile([HD, DIM], MM_DT, tag=f"wp{h}", name=f"wp{h}") for h in range(HPC)]
        for h in range(HPC):
            nc.sync.dma_start(out=wp_sb[h], in_=wpT_d[h * HD:(h + 1) * HD, :])

        # v extended to 97 cols: 80 v-dims, 16 zero pad, ones col at 96 so the
        # denominator lands on a 32-aligned PSUM partition. Split per 512-s
        # superchunk so attention can start before all of phase A finishes.
        VEXT = 97
        v_sb = [[per.tile([128, 4, VEXT], MM_DT, tag=f"v{h}_{ss}",
                          name=f"v{h}_{ss}") for ss in range(NSS)]
                for h in range(HPC)]
        qkT = [[per.tile([HD, 512], MM_DT, tag=f"qkT{j}_{ss}",
                         name=f"qkT{j}_{ss}") for ss in range(NSS)]
               for j in range(2 * HPC)]
        att_o = [[per.tile([HD, 512], MM_DT, tag=f"atto{h}_{ss}",
                           name=f"atto{h}_{ss}") for ss in range(NSS)]
                 for h in range(HPC)]

        # one shared PSUM pool (8 bank-sized slots shared by every phase so
        # the scheduler can overlap A/B/C), plus top-level SBUF pools
        psp = ctx.enter_context(tc.tile_pool(name="ps", bufs=8, space="PSUM"))
        xtp = ctx.enter_context(tc.tile_pool(name="xt", bufs=10))
        stgp = ctx.enter_context(tc.tile_pool(name="stg", bufs=6))
        ropep = ctx.enter_context(tc.tile_pool(name="ropet", bufs=4))
        qkrop = ctx.enter_context(tc.tile_pool(name="qkro", bufs=4))
        expp = ctx.enter_context(tc.tile_pool(name="expp", bufs=14))
        smp = ctx.enter_context(tc.tile_pool(name="smalls", bufs=4))
        outp = ctx.enter_context(tc.tile_pool(name="outp", bufs=2))

        if nbm:
            bmask_sb = per.tile([128, nbm], MM_DT, tag="bmask")
            nc.sync.dma_start(out=bmask_sb,
                              in_=bmask_d.ap().rearrange("n p -> p n"))
            bidx = {(j, r0, r1): i for i, (j, r0, r1) in enumerate(bpairs)}

        # ---------------- phase bodies (emitted interleaved below) --------
        nh = 2 * HPC  # 4 roped qk tensor-heads

        def a_quanta(ss):
            """QKV + RoPE + transposes for s-superchunk ss, as a list of
            small emission quanta so B/C work can interleave between them."""
            st = {}

            def header():
                xts = []
                for dp in range(5):
                    if ss == 0:
                        for d in (2 * dp, 2 * dp + 1):
                            nc.sync.dma_start(
                                out=wqkv_sb[d],
                                in_=wqkvT_d[128 * d:128 * (d + 1), :])
                    xt = xtp.tile([128, 2, 512], MM_DT, tag="xt", name="xt")
                    nc.sync.dma_start(out=xt, in_=xblk_d[ss, dp])
                    xts.append(xt)
                nc.sync.dma_start(out=cos_sb[ss], in_=cosb_d[ss])
                nc.sync.dma_start(out=sin_sb[ss], in_=sinb_d[ss])
                for h in range(HPC):
                    nc.sync.dma_start(
                        out=v_sb[h][ss][:, :, HD:VEXT],
                        in_=bass.AP(tensor=vpad_d, offset=0,
                                    ap=[[0, 128], [0, 4], [1, VEXT - HD]]))
                st["xts"] = xts
                # all 16 sub-transposes packed into 2 PSUM banks (not 4) so
                # phase B keeps enough free banks to run ahead
                st["tp"] = [psp.tile([HD, 4, 2, 128], MM_DT, tag="ps",
                                     name="tpps") for _ in range(HPC)]

            def half_qkT_copy(half):
                # copy transposed qk halves out as soon as both their subs
                # are emitted: B quanta touching only this half can then
                # start without waiting for the rest of the superchunk
                for j in range(2 * HPC):
                    nc.vector.tensor_copy(
                        qkT[j][ss][:, 256 * half:256 * (half + 1)]
                        .rearrange("p (s c) -> p s c", s=2),
                        st["tp"][j // 2][:, 2 * half:2 * half + 2, j % 2, :])

            def transposes(sub):
                # transpose roped q,k of `sub` to [hd, s]; emitted one sub
                # late so the rope chain hides behind the next sub's matmuls
                ro = st.pop(f"ro{sub}")
                for j in range(2 * HPC):
                    nc.tensor.transpose(
                        st["tp"][j // 2][:, sub, j % 2, :],
                        ro[:, HD * j:HD * (j + 1)], ident_sb)
                if sub == 1:
                    half_qkT_copy(0)
                elif sub == 3:
                    half_qkT_copy(1)

            def sub_q(sub):
                def f():
                    xts = st["xts"]
                    qp = psp.tile([128, ODIM], F32, tag="ps", name="qkvps")
                    for d in range(10):
                        nc.tensor.matmul(
                            qp[:, :],
                            lhsT=xts[d // 2][:, d % 2,
                                             128 * sub:128 * (sub + 1)],
                            rhs=wqkv_sb[d], start=(d == 0), stop=False)
                    nc.tensor.matmul(qp[:, :], lhsT=ones_sb[:, :],
                                     rhs=bqkv_sb[:, :], start=False, stop=True)
                    if sub > 0:
                        transposes(sub - 1)

                    # single PSUM->SBUF staging copy (DVE); rope + v copies
                    # then run on the otherwise-idle Pool engine (Pool cannot
                    # touch PSUM, so the stage hop is what unlocks it)
                    stg = stgp.tile([128, ODIM], F32, tag="stg", name="stg")
                    nc.vector.tensor_copy(stg, qp)

                    # RoPE over q,k: out = t*cos + halfswap(t)*sinsgn
                    m1 = ropep.tile([128, QKDIM], F32, tag="m1")
                    m2 = ropep.tile([128, QKDIM], F32, tag="m2")
                    qk_h = stg[:, 0:QKDIM].rearrange("p (h d) -> p h d", h=nh)
                    cos_b = cos_sb[ss][:, sub:sub + 1, :].to_broadcast(
                        [128, nh, HD])
                    nc.gpsimd.tensor_mul(
                        m1.rearrange("p (h d) -> p h d", h=nh), qk_h, cos_b)
                    swap = stg[:, 0:QKDIM].rearrange(
                        "p (h x d) -> p h x d", h=nh, x=2)[:, :, ::-1, :]
                    sin_b = sin_sb[ss][:, sub:sub + 1, :].rearrange(
                        "p t (x d) -> p (t x) d", x=2)[:, None, :, :] \
                        .to_broadcast([128, nh, 2, HD // 2])
                    nc.gpsimd.tensor_mul(
                        m2.rearrange("p (h x d) -> p h x d", h=nh, x=2),
                        swap, sin_b)
                    ro = qkrop.tile([128, QKDIM], MM_DT, tag="qkro")
                    with nc.allow_low_precision("bf16 matmul inputs"):
                        nc.gpsimd.tensor_add(ro, m1, m2)
                    st[f"ro{sub}"] = ro

                    # v natural copy (its bias already in psum)
                    for h in range(HPC):
                        with nc.allow_low_precision("bf16 matmul inputs"):
                            nc.gpsimd.tensor_copy(
                                v_sb[h][ss][:, sub, 0:HD],
                                stg[:, QKDIM + HD * h:QKDIM + HD * (h + 1)])
                return f

            def last_tp():
                transposes(3)

            return [header] + [sub_q(s) for s in range(4)] + [last_tp]

        # C-superchunk readiness bookkeeping (emission-order only; runtime
        # ordering is enforced by tile dependencies)
        from collections import deque
        ready: deque = deque()
        need = {sc: 0 for sc in range(NSS)}
        got = {sc: 0 for sc in range(NSS)}

        def seg_qchunks(seg):
            s0, s1 = seg
            g = (s0 // 512) * 512
            out = []
            while g < s1:
                q0, q1 = max(s0, g), min(s1, g + 512)
                if q1 > q0:
                    out.append((q0, q1))
                g += 512
            return out

        def b_quantum(seg, h, chunks):
            """attention for one (segment, head) over `chunks` q-chunks:
            chunks processed together share each k/v stationary block (one
            LdWeights per block); av of block j-1 is emitted between scores
            and av of block j so exp latency hides behind PE work."""
            s0, s1 = seg
            jb0, jb1 = s0 // 128, -(-s1 // 128)
            qcs = []
            for (q0, q1) in chunks:
                qn = q1 - q0
                ss_q, c0 = q0 // 512, q0 % 512
                # matmul moving dim kept 4-aligned: widen the compute window
                # to 4-aligned columns (scratch cols unread)
                qa0 = q0 - (q0 % 4)
                qa1 = min(512 * (ss_q + 1), q1 + ((-q1) % 4))
                qcs.append((q0, q1, qn, ss_q, c0,
                            qa1 - qa0, q0 - qa0, qa0 % 512))
            st = {"ap": [], "den": []}

            def body():
                assert len(qcs) == 1, "one PSUM accumulation group at a time"
                (q0, q1, qn, ss_q, c0, qna, off, ca0) = qcs[0]
                ap_ = psp.tile([VEXT, 512], F32, tag="ps", name="attps")
                st["ap"].append(ap_)
                blocks = list(range(jb0, jb1))
                for g0 in range(0, len(blocks), 3):
                    grp = blocks[g0:g0 + 3]
                    exs = []
                    for j in grp:
                        kTt = qkT[HPC + h][j // 4]
                        sc = psp.tile([128, 512], F32, tag="ps",
                                      name="scps")
                        nc.tensor.matmul(
                            sc[:, :qna],
                            lhsT=kTt[:, 128 * (j % 4):128 * (j % 4 + 1)],
                            rhs=qkT[h][ss_q][:, ca0:ca0 + qna],
                            start=True, stop=True)
                        ex = expp.tile([128, 512], MM_DT, tag="expp")
                        with nc.allow_low_precision("bf16 matmul inputs"):
                            nc.scalar.activation(
                                ex[:, :qna], sc[:, :qna],
                                mybir.ActivationFunctionType.Exp)
                        r0, r1 = max(0, s0 - 128 * j), min(128, s1 - 128 * j)
                        if r0 > 0 or r1 < 128:
                            # zero out-of-segment rows of this block
                            mi = bidx[(j, r0, r1)]
                            with nc.allow_low_precision("f32r inputs"):
                                nc.vector.tensor_mul(
                                    ex[:, :qna], ex[:, :qna],
                                    bmask_sb[:, mi:mi + 1]
                                    .to_broadcast([128, qna]))
                        exs.append(ex)
                    for j, ex in zip(grp, exs):
                        nc.tensor.matmul(
                            ap_[:, :qna],
                            lhsT=v_sb[h][j // 4][:, j % 4, :],
                            rhs=ex[:, :qna],
                            start=(j == blocks[0]),
                            stop=(j == blocks[-1]))
                # the denominator rows leave PSUM right away; the rest of
                # the normalize tail is emitted later (after other PE work)
                for ci, (q0, q1, qn, ss_q, c0, qna, off, ca0) in \
                        enumerate(qcs):
                    den = smp.tile([1, 512], MM_DT, tag="den", name="den")
                    with nc.allow_low_precision("bf16 matmul inputs"):
                        nc.vector.tensor_copy(den[:, :qna],
                                              st["ap"][ci][96:97, :qna])
                    st["den"].append(den)

            def tail():
                recs = []
                for ci, (q0, q1, qn, ss_q, c0, qna, off, ca0) in \
                        enumerate(qcs):
                    bc = psp.tile([HD, 512], F32, tag="ps", name="bcps")
                    nc.tensor.matmul(bc[:, :qna], lhsT=ones_sb[:, 0:HD],
                                     rhs=st["den"][ci][:, :qna],
                                     start=True, stop=True)
                    rec = smp.tile([HD, 512], F32, tag="rec", name="rec")
                    nc.vector.reciprocal(rec[:, :qna], bc[:, :qna])
                    recs.append(rec)
                for ci, (q0, q1, qn, ss_q, c0, qna, off, ca0) in \
                        enumerate(qcs):
                    with nc.allow_low_precision("f32r matmul inputs"):
                        nc.vector.tensor_mul(
                            att_o[h][ss_q][:, c0:c0 + qn],
                            st["ap"][ci][0:HD, off:off + qn],
                            recs[ci][:, off:off + qn])
                    got[ss_q] += 1
                    if got[ss_q] == need[ss_q]:
                        ready.extend(c_quanta(ss_q))
            return body, tail

        def c_quanta(sc_):
            """projection for output s-superchunk sc_, one quantum per
            128-dim half. b_proj is added host-side after the partial sum."""
            def mk(mh):
                def f():
                    ob = outp.tile([128, 5, 512], MM_DT, tag="outp")
                    for mm_ in range(5):
                        m = 5 * mh + mm_
                        pp = psp.tile([128, 512], F32, tag="ps", name="prps")
                        for h in range(HPC):
                            nc.tensor.matmul(
                                pp[:, :],
                                lhsT=wp_sb[h][:, 128 * m:128 * (m + 1)],
                                rhs=att_o[h][sc_],
                                start=(h == 0), stop=(h == HPC - 1))
                        with nc.allow_low_precision("bf16 output partials"):
                            if mm_ % 2 == 0:
                                nc.scalar.copy(ob[:, mm_, :], pp)
                            else:
                                nc.vector.tensor_copy(ob[:, mm_, :], pp)
                    nc.sync.dma_start(out=outb_d[sc_, mh], in_=ob)
                return f
            return [mk(0), mk(1)]

        # ---- interleaved driver: A emitted in sub-sized quanta; pending
        # ---- B/C quanta drain between them so PE never sits on one phase.
        # ---- B normalize tails are deferred one quantum (the next body's
        # ---- matmuls hide the den-copy latency on the in-order PE stream).
        for seg in segments:
            for (q0, q1) in seg_qchunks(seg):
                need[q0 // 512] += HPC
        pend = [None]

        def make_b(seg, h, chunks):
            body, tail = b_quantum(seg, h, chunks)

            def run():
                body()
                if pend[0] is not None:
                    pend[0]()
                pend[0] = tail
            return run

        segs_left = sorted(segments, key=lambda s: s[1])
        for ss in range(NSS):
            for q in a_quanta(ss):
                q()
                for _ in range(2 if len(ready) > 3 else 1):
                    if ready:
                        ready.popleft()()
            done_to = 512 * (ss + 1)
            while segs_left and segs_left[0][1] <= done_to:
                seg = segs_left.pop(0)
                for h in range(HPC):
                    if B_MERGE:
                        ready.append(make_b(seg, h, seg_qchunks(seg)))
                    else:
                        for qc in seg_qchunks(seg):
                            ready.append(make_b(seg, h, [qc]))
        assert not segs_left
        while ready or pend[0] is not None:
            if ready:
                ready.popleft()()
            else:
                t, pend[0] = pend[0], None
                t()

    nc.compile()
    return nc


def _prep_inputs(x, cu_seqlens, rotary_pos_emb, w_qkv, b_qkv, w_proj, b_proj):
    """Host-side shard prep. Returns per-core input dicts."""
    import ml_dtypes
    mm_np = ml_dtypes.bfloat16
    scale = np.float32(1.0 / np.sqrt(np.float32(HD)))
    xT = np.ascontiguousarray(np.asarray(x, np.float32).T)
    w_qkv = np.asarray(w_qkv, np.float32)
    b_qkv = np.asarray(b_qkv, np.float32)
    w_proj = np.asarray(w_proj, np.float32)
    b_proj = np.asarray(b_proj, np.float32)
    rot = np.asarray(rotary_pos_emb, np.float32)

    cosw = np.concatenate([np.cos(rot), np.cos(rot)], axis=1).astype(np.float32)
    sinw = np.concatenate([-np.sin(rot), np.sin(rot)], axis=1).astype(np.float32)
    # blocked layouts so every device DMA reads one dense contiguous region:
    # xblk[ss,dp,p,c,n] = xT[256dp+128c+p, 512ss+n]; cosb[ss,p,t,d] likewise
    xblk = np.ascontiguousarray(
        xT.reshape(5, 2, 128, 6, 512).transpose(3, 0, 2, 1, 4)).astype(mm_np)
    cosb = np.ascontiguousarray(
        cosw.reshape(6, 4, 128, HD).transpose(0, 2, 1, 3))
    sinb = np.ascontiguousarray(
        sinw.reshape(6, 4, 128, HD).transpose(0, 2, 1, 3))
    ident = np.eye(128, dtype=mm_np)
    onesrow = np.ones((1, 128), dtype=mm_np)
    vpad = np.zeros(17, dtype=mm_np)
    vpad[16] = 1.0
    segments = _segments_from_cu(cu_seqlens)
    bmask_rows = []
    for (s0, s1) in segments:
        for j in range(s0 // 128, -(-s1 // 128)):
            r0, r1 = max(0, s0 - 128 * j), min(128, s1 - 128 * j)
            if r0 > 0 or r1 < 128:
                row = np.zeros(128, dtype=mm_np)
                row[r0:r1] = 1.0
                bmask_rows.append(row)
    bmask = np.stack(bmask_rows) if bmask_rows else None

    in_maps = []
    for c in range(NCORES):
        heads = [HPC * c + i for i in range(HPC)]
        idx = []
        for base in (0, DIM, 2 * DIM):           # q, k, v row blocks
            for h in heads:
                idx.extend(range(base + h * HD, base + (h + 1) * HD))
        w_c = w_qkv[idx, :].copy()
        b_c = b_qkv[idx].copy()
        w_c[:QKDIM // 2] *= scale                # scale q by 1/sqrt(HD)
        b_c[:QKDIM // 2] *= scale
        cdims = []
        for h in heads:
            cdims.extend(range(h * HD, (h + 1) * HD))
        wpT = np.ascontiguousarray(w_proj[:, cdims].T).astype(mm_np)  # [CDIM, DIM]
        in_maps.append({
            "xblk": xblk,
            "wqkvT": np.ascontiguousarray(w_c.T).astype(mm_np),
            "bqkv": np.ascontiguousarray(b_c[None, :]).astype(mm_np),
            "cosb": cosb,
            "sinb": sinb,
            "wpT": wpT,
            "ident": ident,
            "onesrow": onesrow,
            "vpad": vpad,
        })
        if bmask is not None:
            in_maps[-1]["bmask"] = bmask
    return in_maps


def run(inputs: dict, trace: bool = False):
    """Build (cached), run on 8 cores, return (out [S, DIM] fp32, results)."""
    segments = _segments_from_cu(inputs["cu_seqlens"])
    key = (segments, str(MM_DT))
    if key not in _CACHE:
        _CACHE[key] = _build(segments)
    nc = _CACHE[key]
    in_maps = _prep_inputs(
        inputs["x"], inputs["cu_seqlens"], inputs["rotary_pos_emb"],
        inputs["w_qkv"], inputs["b_qkv"], inputs["w_proj"], inputs["b_proj"])
    res = run_bass_kernel_spmd(nc, in_maps, core_ids=list(range(NCORES)),
                               trace=trace)
    acc = np.zeros((DIM, S), np.float64)
    for r in res.results:
        # blocked [sc, mh, p, c, n] -> [dim = 640*mh+128*c+p, s = 512*sc+n]
        acc += r["outb"].astype(np.float32) \
            .transpose(1, 3, 2, 0, 4).reshape(DIM, S)
    acc += np.asarray(inputs["b_proj"], np.float64)[:, None]
    out = np.ascontiguousarray(acc.T.astype(np.float32))
    return out, res


def kernel(**inputs) -> np.ndarray:
    out, _ = run(inputs, trace=False)
    return out

